# revision 13
# baseline (speedup 1.0000x reference)
"""Trainium2 Bass kernel for ViT-style multi-head attention with relative
position bias.

Problem (per full input):
  x        [8, 1024, 768] f32
  W_qkv    [768, 2304]    f32
  W_proj   [768, 768]     f32
  b_proj   [768]          f32
  bias_table [2047, 12]   f32
  rel_index  [1024, 1024] int32

Sharding: pure data parallel - one batch element per NeuronCore (B=8 over 8
cores), weights replicated. No collectives.

v3 design (vs the ~158us v2): the v2 steady state is paced by the ACT
engine doing ALL 96 softmax exps (1.15us each) while the DVE carries the
skew-multiplies + all PSUM->SBUF casts (~125us) - three engines each near
~110-136us. v3 rebalances the per-(jc,it) piece work across four engines:

  - exp SPLIT: pieces in SCH_SET run a fused Schraudolph fast-exp on the
    DVE: ONE scalar_tensor_tensor computes int16(sc*A16 + T16[j-i]) whose
    int16 bits ARE the bf16 pattern of exp(sc + t[j-i]) (bias add, exp,
    and bf16 store fused; ~1.23us/piece, PSUM fp32 read at 1x). The
    remaining pieces use the exact ACT exp + skew multiply as before.
    Softmax normalization cancels the Schraudolph sawtooth's common mode;
    measured sim rel-err 1.12e-2 at a 6/16 split, insensitive to the
    DVE's fp32->int16 rounding mode.
  - the skew multiplies for ACT-path pieces move mostly to GPSIMD
    (idle in v2; ~2.2us/piece vs 0.68 on DVE, but it's free capacity).
  - qkv PSUM->SBUF casts move to ACT (Copy activation, fits ACT slack).
  - two host tables per pair stream one phase ahead: bf16 multiplier
    exp(t) for ACT pieces (it=0 window u in [0,1408)) and int16
    T16 = rint(A16*t + 16256 - C16) for DVE pieces (it=1 window
    u0=512).
  - lead-in: xT loads as 6 per-kc chunk tiles so qk(0) starts on chunk 0
    (~3us) instead of waiting for all of xT (~18us in v2).

Everything else (row-tiled concurrent K=64 scores, flipped PV with ones
column, PE transposes, pipelined tail) is unchanged from v2.
"""

import numpy as np
import ml_dtypes

B = 8
N = 1024
C = 768
H = 12
DH = 64
P = 128
KC = C // P          # 6 contraction chunks of 128 over C
NJ = N // P          # 8 chunks of 128 over the j (key) axis
NT = N // 512        # 2 tiles of 512 over the i (query) axis
HP = H // 2          # 6 head pairs
T5 = 512
SW = 1408            # windowed skew-table width (per path)
A16 = 128.0 / float(np.log(2.0))   # schraudolph exponent scale for bf16
C16 = 5.0                          # schraudolph centering constant

# pieces (jc, it) whose exp runs as fused Schraudolph on the DVE.
# ALL it=1 pieces: every jc then runs [ACT exp it0 || DVE STT it1]
# concurrently, with no double-ACT jc's to wrinkle the sc PSUM ring.
# (it=1 with jc < 4 could not use the bf16 table window anyway.)
SCH_SET = {(jc, 1) for jc in range(8)}
# ACT-path (it=0) pieces whose skew multiply runs on GPSIMD (rest on DVE)
MUL_GPS = {(0, 0), (1, 0), (2, 0), (3, 0), (5, 0), (7, 0)}

_BUILT = {}


def _build_nc():
    from contextlib import ExitStack
    import concourse.bass as bass
    import concourse.mybir as mybir
    import concourse.tile as tile
    from concourse import bacc
    from concourse import masks

    bf16 = mybir.dt.bfloat16
    i16 = mybir.dt.int16
    f32 = mybir.dt.float32
    Exp = mybir.ActivationFunctionType.Exp
    MUL = mybir.AluOpType.mult
    ADD = mybir.AluOpType.add

    nc = bacc.Bacc("TRN2", target_bir_lowering=False, debug=False)

    xT_d = nc.dram_tensor("xT", [C, N], bf16, kind="ExternalInput")
    w_d = nc.dram_tensor("wqk", [HP, P, KC, 2 * P], bf16, kind="ExternalInput")
    wv_d = nc.dram_tensor("wv", [C, C], bf16, kind="ExternalInput")
    wp_d = nc.dram_tensor("wproj", [C, C], bf16, kind="ExternalInput")
    bp_d = nc.dram_tensor("bproj", [C], f32, kind="ExternalInput")
    skb_d = nc.dram_tensor("skb", [HP, 2, P, SW], bf16, kind="ExternalInput")
    ski_d = nc.dram_tensor("ski", [HP, 2, P, SW], i16, kind="ExternalInput")
    out_d = nc.dram_tensor("out", [N, C], f32, kind="ExternalOutput")

    with ExitStack() as ctx:
        tc = ctx.enter_context(tile.TileContext(nc))

        singles = ctx.enter_context(tc.tile_pool(name="singles", bufs=1))
        pt_pool = ctx.enter_context(tc.tile_pool(name="pt_pool", bufs=2))
        sb_pool = ctx.enter_context(tc.tile_pool(name="sb_pool", bufs=2))
        si_pool = ctx.enter_context(tc.tile_pool(name="si_pool", bufs=2))
        es_pool = ctx.enter_context(tc.tile_pool(name="es_pool", bufs=6))
        on_pool = ctx.enter_context(tc.tile_pool(name="on_pool", bufs=6))
        rec_pool = ctx.enter_context(tc.tile_pool(name="rec_pool", bufs=6))
        ost_pool = ctx.enter_context(tc.tile_pool(name="ost_pool", bufs=2))
        sc_ps = ctx.enter_context(tc.tile_pool(name="sc_ps", bufs=2, space="PSUM"))
        mm_ps = ctx.enter_context(tc.tile_pool(name="mm_ps", bufs=2, space="PSUM"))
        pv_ps = ctx.enter_context(tc.tile_pool(name="pv_ps", bufs=2, space="PSUM"))

        # ---- resident SBUF tensors ----
        # xT as 6 per-kc chunk tiles: qk(0) kc-loop consumes them as the
        # 256KB chunk DMAs land instead of waiting for the whole 1.5MB
        xT_r = xT_d.ap().rearrange("(kc p) n -> p kc n", p=P)
        xc = [singles.tile([P, N], bf16, name=f"xc{k}") for k in range(KC)]
        w_pairs = [singles.tile([P, KC, 2 * P], bf16, name=f"w_pair{hp}")
                   for hp in range(HP)]
        wv_sb = singles.tile([P, KC, C], bf16)
        wv_r = wv_d.ap().rearrange("(kc p) d -> p kc d", p=P)
        wp_sb = singles.tile([P, KC, C], bf16)
        bp_sb = singles.tile([P, C], f32)

        sb_tiles = [None] * HP
        si_tiles = [None] * HP

        def prefetch_skew(hp, eng=None):
            sb_tiles[hp] = sb_pool.tile([P, 2, SW], bf16, tag="skb",
                                        name=f"skb_{hp}")
            si_tiles[hp] = si_pool.tile([P, 2, SW], i16, tag="ski",
                                        name=f"ski_{hp}")
            e = eng or nc.sync
            e.dma_start(out=sb_tiles[hp],
                        in_=skb_d.ap()[hp].rearrange("t p u -> p t u"))
            e.dma_start(out=si_tiles[hp],
                        in_=ski_d.ap()[hp].rearrange("t p u -> p t u"))

        # lead-critical bytes: the first qk group consumes (xc[kc], w0[kc])
        # in kc order, so interleave per-kc chunks of both across the
        # sync+scalar rings. Pair-0's skew tables would otherwise queue
        # behind all of this on sync and stall phase 0's softmax, so they
        # go out first on the gpsimd SWDGE ring (concurrent queues).
        prefetch_skew(0, eng=nc.gpsimd)
        nc.sync.dma_start(out=xc[0], in_=xT_r[:, 0, :])
        nc.scalar.dma_start(out=w_pairs[0][:, 0, :], in_=w_d.ap()[0][:, 0, :])
        nc.scalar.dma_start(out=xc[1], in_=xT_r[:, 1, :])
        nc.sync.dma_start(out=xc[2], in_=xT_r[:, 2, :])
        for k in range(1, 3):
            nc.scalar.dma_start(out=w_pairs[0][:, k, :], in_=w_d.ap()[0][:, k, :])
        nc.scalar.dma_start(out=xc[3], in_=xT_r[:, 3, :])
        nc.sync.dma_start(out=xc[4], in_=xT_r[:, 4, :])
        for k in range(3, 6):
            nc.scalar.dma_start(out=w_pairs[0][:, k, :], in_=w_d.ap()[0][:, k, :])
        nc.sync.dma_start(out=xc[5], in_=xT_r[:, 5, :])
        nc.sync.dma_start(out=wv_sb, in_=wv_r)
        for hp in range(1, HP):
            nc.sync.dma_start(out=w_pairs[hp], in_=w_d.ap()[hp])
        nc.sync.dma_start(out=wp_sb, in_=wp_d.ap().rearrange("(kc p) d -> p kc d", p=P))

        ident = singles.tile([P, P], bf16)
        masks.make_identity(nc, ident[:, :])

        qT_sb = singles.tile([P, HP, N], bf16)   # chunk hp = heads (2hp, 2hp+1)
        kT_sb = singles.tile([P, HP, N], bf16)
        v_sb = singles.tile([P, NJ, H, DH + 1], bf16)  # col DH = ones
        nc.vector.memset(v_sb[:, :, :, DH:DH + 1], 1.0)
        oT_sb = singles.tile([P, KC, N], bf16)   # kc chunk == pair hp

        # ---- matmul group emitters ----

        def v_group(nj, et):
            e0 = et * 384
            def emit():
                ps = mm_ps.tile([P, 384], f32, tag="mm", name=f"ps_v_{nj}_{et}")
                for kc in range(KC):
                    nc.tensor.matmul(
                        ps,
                        xc[kc][:, nj * P:(nj + 1) * P],
                        wv_sb[:, kc, e0:e0 + 384],
                        start=(kc == 0), stop=(kc == KC - 1),
                    )
                h0 = e0 // DH
                nc.scalar.copy(
                    out=v_sb[:, nj, h0:h0 + 6, 0:DH],
                    in_=ps.rearrange("p (h d) -> p h d", h=6),
                )
            return emit

        def qk_group(hp, which, it):
            col0 = which * P
            def emit():
                d = qT_sb if which == 0 else kT_sb
                ps = mm_ps.tile([P, T5], f32, tag="mm",
                                name=f"ps_qk_{hp}_{which}_{it}")
                for kc in range(KC):
                    nc.tensor.matmul(
                        ps,
                        w_pairs[hp][:, kc, col0:col0 + P],
                        xc[kc][:, it * T5:(it + 1) * T5],
                        start=(kc == 0), stop=(kc == KC - 1),
                    )
                nc.scalar.copy(out=d[:, hp, it * T5:(it + 1) * T5], in_=ps)
            return emit

        # pending (hp, ib, o_n) tiles awaiting PE transpose into oT_sb
        pend = []

        def pv_group(hp, ib):
            """Flipped PV for both heads of pair hp, query block ib."""
            def emit():
                pt = pts[hp]
                pv = pv_ps.tile([P, 2, DH + 1], f32, tag="pv",
                                name=f"pv_{hp}_{ib}")
                for t in range(2):
                    h = 2 * hp + t
                    for jc in range(NJ):
                        nc.tensor.matmul(
                            pv[:, t, :],
                            pt[:, jc, t, ib * P:(ib + 1) * P],
                            v_sb[:, jc, h, :],
                            start=(jc == 0), stop=(jc == NJ - 1),
                        )
                rec = rec_pool.tile([P, 2, 1], f32, tag="rec",
                                    name=f"rec_{hp}_{ib}")
                nc.vector.reciprocal(rec, pv[:, :, DH:DH + 1])
                o_n = on_pool.tile([P, 2, DH], bf16, tag="on",
                                   name=f"on_{hp}_{ib}")
                nc.vector.tensor_mul(
                    out=o_n,
                    in0=pv[:, :, 0:DH],
                    in1=rec[:, :, :].broadcast_to([P, 2, DH]),
                )
                pend.append((hp, ib, o_n))
            return emit

        def flush_tp(lag=1):
            while len(pend) > lag:
                hp, ib, o_n = pend.pop(0)
                tp = mm_ps.tile([P, P], bf16, tag="mm", name=f"tp_{hp}_{ib}")
                nc.tensor.transpose(
                    tp, o_n[:, :, :].rearrange("p t d -> p (t d)"), ident)
                nc.scalar.copy(
                    out=oT_sb[:, hp, ib * P:(ib + 1) * P], in_=tp)

        def proj_group(nj):
            def emit():
                osb = ost_pool.tile([P, C], f32, tag="osb",
                                    name=f"osb_{nj}")
                for et in range(2):
                    pp = sc_ps.tile([P, 2, T5], f32, tag="sc",
                                    name=f"pp_{nj}_{et}")[:, 0, 0:384]
                    for kc in range(KC):
                        nc.tensor.matmul(
                            pp,
                            oT_sb[:, kc, nj * P:(nj + 1) * P],
                            wp_sb[:, kc, et * 384:(et + 1) * 384],
                            start=(kc == 0), stop=(kc == KC - 1),
                        )
                    nc.vector.tensor_add(
                        out=osb[:, et * 384:(et + 1) * 384],
                        in0=pp,
                        in1=bp_sb[:, et * 384:(et + 1) * 384],
                    )
                nc.sync.dma_start(
                    out=out_d.ap()[nj * P:(nj + 1) * P, :], in_=osb)
            return emit

        # ---- scores: per-(jc, it) piece, engine-split softmax ----

        def scores_piece(hp, pt, jc, it):
            off = 896 - 128 * jc
            sc = sc_ps.tile([P, 2, T5], f32, tag="sc",
                            name=f"sc_{hp}_{jc}_{it}")
            nc.tensor.matmul(
                sc[:, 0, :],
                kT_sb[0:DH, hp, jc * P:(jc + 1) * P],
                qT_sb[0:DH, hp, it * T5:(it + 1) * T5],
                start=True, stop=True,
            )
            nc.tensor.matmul(
                sc[:, 1, :],
                kT_sb[DH:P, hp, jc * P:(jc + 1) * P],
                qT_sb[DH:P, hp, it * T5:(it + 1) * T5],
                start=True, stop=True,
            )
            dst = pt[:, jc, :, it * T5:(it + 1) * T5]
            if (jc, it) in SCH_SET:
                # fused fast-exp: int16(sc*A16 + T16) bits == bf16 of
                # exp(sc + t); T16 window starts at u0=512 so the it=1
                # slice is [off, off+512)
                nc.vector.scalar_tensor_tensor(
                    out=dst.bitcast(i16),
                    in0=sc,
                    scalar=A16,
                    in1=si_tiles[hp][:, :, off:off + T5],
                    op0=MUL,
                    op1=ADD,
                )
            else:
                es = es_pool.tile([P, 2, T5], bf16, tag="es",
                                  name=f"es_{hp}_{jc}_{it}")
                nc.scalar.activation(out=es, in_=sc, func=Exp)
                eng = nc.gpsimd if (jc, it) in MUL_GPS else nc.vector
                eng.tensor_mul(
                    out=dst,
                    in0=es,
                    in1=sb_tiles[hp][:, :, off + it * T5:off + (it + 1) * T5])

        def scores_phase(hp, pt, slots):
            for jc in range(NJ):
                for it in range(NT):
                    scores_piece(hp, pt, jc, it)
                for fn in slots[jc]:
                    fn()

        bp_ap = bp_d.ap()
        bp_bcast = bass.AP(tensor=bp_ap.tensor, offset=bp_ap.offset,
                           ap=[[0, P], *bp_ap.ap])
        nc.gpsimd.dma_start(out=bp_sb, in_=bp_bcast)
        # dummy exp: pulls the ACT table load off the critical path
        dummy = rec_pool.tile([1, 1], f32, tag="rec", name="act_warm")
        nc.scalar.activation(out=dummy, in_=ident[0:1, 0:1], func=Exp)

        for which in range(2):
            for it in range(NT):
                qk_group(0, which, it)()

        pts = [None] * HP
        v0 = [v_group(nj, 0) for nj in range(NJ)]
        v1 = [v_group(nj, 1) for nj in range(NJ)]
        v1_sched = {1: v1[0:3], 2: v1[3:6], 3: v1[6:8]}

        fl = lambda: flush_tp(lag=1)
        prefetch_skew(1)
        pts[0] = pt_pool.tile([P, NJ, 2, N], bf16, tag="pt", name="pt_0")
        slots0 = [[v0[k]] for k in range(NJ)]
        qks0 = [qk_group(1, w, it) for w in range(2) for it in range(NT)]
        slots0[1].append(qks0[0])
        slots0[3].append(qks0[1])
        slots0[5].append(qks0[2])
        slots0[7].append(qks0[3])
        scores_phase(0, pts[0], slots0)

        for hp in range(1, HP):
            if hp + 1 < HP:
                prefetch_skew(hp + 1)
            slots = [[] for _ in range(NJ)]
            if hp + 1 < HP:
                qks = [qk_group(hp + 1, w, it)
                       for w in range(2) for it in range(NT)]
                slots[1].append(qks[0])
                slots[3].append(qks[1])
                slots[5].append(qks[2])
                slots[7].append(qks[3])
            for i, g in enumerate(v1_sched.get(hp, [])):
                slots[2 * i].append(g)
            for k in range(NJ):
                slots[k].insert(0, pv_group(hp - 1, k))
                slots[k].append(fl)
            pts[hp] = pt_pool.tile([P, NJ, 2, N], bf16, tag="pt",
                                   name=f"pt_{hp}")
            scores_phase(hp, pts[hp], slots)

        # tail: PV(last pair) -> transpose -> proj, pipelined per i-block.
        # ACT is idle in the tail, so transpose evicts go there instead of
        # the (busier) DVE, and PV groups run one block ahead of proj.
        def flush_until(hp, ib):
            while pend:
                h0, i0, o_n = pend.pop(0)
                tp = mm_ps.tile([P, P], bf16, tag="mm", name=f"tp_{h0}_{i0}")
                nc.tensor.transpose(
                    tp, o_n[:, :, :].rearrange("p t d -> p (t d)"), ident)
                nc.scalar.copy(
                    out=oT_sb[:, h0, i0 * P:(i0 + 1) * P], in_=tp)
                if h0 == hp and i0 == ib:
                    break

        # PE-warm bridge: after the last scores the PE idles >3.4us
        # waiting on the trailing softmax pieces + normalize chain, which
        # trips the HAM MID window and runs the tail at 1.2GHz (measured
        # 10us cold). Dependency-free identity transposes keep it warm.
        for wk in range(24):
            jtp = mm_ps.tile([P, P], bf16, tag="mm", name=f"warm_{wk}")
            nc.tensor.transpose(jtp, ident, ident)

        pv_group(HP - 1, 0)()
        pv_group(HP - 1, 1)()
        for ib in range(2, NJ):
            pv_group(HP - 1, ib)()
            flush_until(HP - 1, ib - 2)
            proj_group(ib - 2)()
        for ib in range(NJ - 2, NJ):
            flush_until(HP - 1, ib)
            proj_group(ib)()

    nc.finalize()
    return nc


def _get_nc():
    if "nc" not in _BUILT:
        _BUILT["nc"] = _build_nc()
    return _BUILT["nc"]


def _prep_inputs(x, W_qkv, W_proj, b_proj, bias_table, rel_index):
    bf = ml_dtypes.bfloat16
    x = np.asarray(x, dtype=np.float32)
    W_qkv = np.asarray(W_qkv, dtype=np.float32)
    W_proj = np.asarray(W_proj, dtype=np.float32)
    b_proj = np.asarray(b_proj, dtype=np.float32)
    bias_table = np.asarray(bias_table, dtype=np.float32)

    xT = np.ascontiguousarray(x.transpose(0, 2, 1)).astype(bf)       # [B, C, N]
    wq = W_qkv.copy()
    wq[:, :C] *= DH ** -0.5          # fold the attention scale into W_q
    Wq = wq[:, :C].reshape(C, HP, DH * 2)
    Wk = wq[:, C:2 * C].reshape(C, HP, DH * 2)
    pairW = np.concatenate([Wq, Wk], axis=2)          # [C, HP, 2P]
    pairW = pairW.reshape(KC, P, HP, 2 * P).transpose(2, 1, 0, 3)
    wqk = np.ascontiguousarray(pairW).astype(bf)      # [HP, P, KC, 2P]
    wv = np.ascontiguousarray(wq[:, 2 * C:]).astype(bf)
    wp = W_proj.astype(bf)

    # windowed skew tables. it=0 pieces read u in [off, off+512) with
    # off = 896-128*jc in [0, 896] -> bf16 multiplier table over
    # u in [0, 1408): skb[h, p, u] = exp(t_h[p - u + 1919]).
    # it=1 pieces read u' in [off+512, off+1024) subset [512, 1920):
    # int16 table stored with u0 = 512:
    # ski[h, p, u] = rint(A16 * t_h[p - (u+512) + 1919] + 16256 - C16)
    pcol = np.arange(P)[:, None]
    ucol = np.arange(SW)[None, :]
    idx_b = pcol - ucol + 1919                       # [P, SW] in [512, 2046]
    t_exp = np.exp(bias_table)                       # [2047, H]
    skb = t_exp[idx_b, :]                            # [P, SW, H]
    skb = np.ascontiguousarray(skb.transpose(2, 0, 1))  # [H, P, SW]
    skb = skb.reshape(HP, 2, P, SW).astype(bf)

    idx_i = pcol - ucol + 1407                       # [P, SW] in [0, 1534]
    t16 = np.rint(A16 * bias_table + (127 * 128 - C16))  # [2047, H]
    ski = t16[idx_i, :]                              # [P, SW, H]
    ski = np.ascontiguousarray(ski.transpose(2, 0, 1))
    ski = ski.reshape(HP, 2, P, SW).astype(np.int16)

    shared = {"wqk": wqk, "wv": wv, "wproj": wp, "bproj": b_proj,
              "skb": skb, "ski": ski}
    in_maps = []
    for b in range(B):
        m = dict(shared)
        m["xT"] = np.ascontiguousarray(xT[b])
        in_maps.append(m)
    return in_maps


def run(x, W_qkv, W_proj, b_proj, bias_table, rel_index, trace=False):
    """Returns (output [B, N, C] f32, exec_time_ns or None)."""
    from concourse.bass_utils import run_bass_kernel_spmd

    nc = _get_nc()
    in_maps = _prep_inputs(x, W_qkv, W_proj, b_proj, bias_table, rel_index)
    res = run_bass_kernel_spmd(nc, in_maps, core_ids=list(range(B)), trace=trace)
    out = np.stack([r["out"] for r in res.results]).astype(np.float32)
    return out, res.exec_time_ns


def kernel(x, W_qkv, W_proj, b_proj, bias_table, rel_index):
    out, _ = run(x, W_qkv, W_proj, b_proj, bias_table, rel_index, trace=False)
    return out
